# revision 21
# baseline (speedup 1.0000x reference)
"""CenterLoss kernel for Trainium2 (8 NeuronCores, data-parallel over batch).

loss = mean_i( ||nx_i||^2 + ||c_{l_i}||^2 - 2*nx_i.c_{l_i} )
     = mean_i( ||nx_i - c_{l_i}||^2 ),  nx_i = x_i / max(||x_i||, EPS)

The reference's (batch, num_classes) distmat is masked down to one column
per row, so only a gather of centers[labels] is needed (memory regime).

Sharding: batch 16384 -> 8 cores x 2048 rows, centers replicated. x row
p*16+j of a core's shard lives at SBUF partition p, free block j.
The centers gather uses InstDMAGatherAnt (gpsimd dma_gather), which is
Q7 descriptor-generation bound (~8.5ns/row on one tx/rx core pair; a
hot-labels probe showed no DRAM-locality sensitivity). Queue q's
desc-gen runs on Q7 core pair q (the plain indirect-DMA path is frozen
to pair 0), so the gather is split across all 4 SWDGE queues; HW
confirmed the pairs run ahead and overlap. The mlp library's ~6us IRAM
load is triggered by the FIRST Q7-executed op after the reload and
contends with concurrent HBM DMAs (10us when racing the x load), so a
tiny memset right after the reload triggers it immediately and signals
T; the x DMA waits for T and is split across the sync+scalar HWDGE
rings with the normalize pipeline run per half. Gather slot i writes
dst[i%128, i//128]; labels are permuted host-side (slot j*128+p =
label of x row p*16+j) and pre-wrapped into the int16 [16, n/16]
layout (replicated to all 8 Q7 core groups). Each gather has its own
semaphore (a shared sem races: its value can mix two gathers'
per-engine completions). A dummy sqrt preloads the ACT table. d = nx-c
and the Square+accumulate reduction are chunked per gather. Raw bacc
with manual semaphores. Each core returns per-partition partial sums;
the host combines.
"""

import numpy as np

B, C, D = 16384, 8192, 64
N_CORES = 8
ROWS = B // N_CORES        # 2048
P = 128
J = ROWS // P              # 16 blocks of D per partition
F = J * D                  # 1024 f32 per partition
NGATH = 4                  # one gather per SWDGE queue / Q7 pair
GBLK = J // NGATH          # 4 J-blocks per gather
H = F // 2                 # x pipeline half size

_CACHE = {}


def _build():
    from contextlib import ExitStack

    import concourse.bass as bass
    from concourse import bacc, library_config, mybir

    nc = bacc.Bacc("TRN2", target_bir_lowering=False, debug=False,
                   num_devices=N_CORES, dynamic_dma_scratch_size=131072,
                   num_swdge_queues=NGATH)
    f32 = mybir.dt.float32
    x = nc.dram_tensor("x", [ROWS, D], f32, kind="ExternalInput").ap()
    labels = nc.dram_tensor("labels", [P, ROWS // 16], mybir.dt.int16,
                            kind="ExternalInput").ap()
    centers = nc.dram_tensor("centers", [C, D], f32,
                             kind="ExternalInput").ap()
    out = nc.dram_tensor("out", [P, NGATH], f32, kind="ExternalOutput").ap()

    with ExitStack() as ctx:
        def sb(n, s, dt=f32):
            return ctx.enter_context(nc.sbuf_tensor(n, s, dt))
        lab_t = sb("lab_t", [P, ROWS // 16], mybir.dt.int16)
        tw = sb("tw", [P, 1])
        x_t = sb("x_t", [P, F])
        c_t = sb("c_t", [P, F])
        xx = sb("xx", [P, F])
        sx = sb("sx", [P, J])
        mn = sb("mn", [P, J])
        inv = sb("inv", [P, J])
        nx = sb("nx", [P, F])
        acc = sb("acc", [P, NGATH])
        L = ctx.enter_context(nc.semaphore("Lsem"))
        X1 = ctx.enter_context(nc.semaphore("X1sem"))
        X2 = ctx.enter_context(nc.semaphore("X2sem"))
        G = [ctx.enter_context(nc.semaphore(f"Gsem{g}")) for g in range(NGATH)]
        A = ctx.enter_context(nc.semaphore("Asem"))   # ACT-produced events
        V = ctx.enter_context(nc.semaphore("Vsem"))   # DVE-produced events

        xr = x.rearrange("(p j) d -> p (j d)", p=P)

        # ---- Sync: labels in, x half 2, result out ----
        nc.sync.dma_start(lab_t[:], labels[:]).then_inc(L, 16)
        nc.sync.dma_start(x_t[:, H:], xr[:, H:]).then_inc(X2, 16)
        nc.sync.wait_ge(A, 4 + NGATH)
        # No final wait on the out DMA: the bacc epilogue's engine drain
        # covers its completion, overlapping the exit barrier ladder.
        nc.sync.dma_start(out, acc[:]).then_inc(L, 16)

        # ---- GpSimd: trigger the mlp IRAM load, then the gathers ----
        nc.gpsimd.load_library(library_config.mlp)
        # First Q7-executed op after the reload faults the library blob in
        # (~2.4us reload + ~6.4us load); doing it with a memset starts the
        # load ~1.5us earlier than the first gather's own pop would.
        nc.gpsimd.memset(tw[:], 0.0)
        nc.gpsimd.wait_ge(L, 16)
        # gather g covers slots/blocks [g*GBLK, (g+1)*GBLK) on SWDGE queue g
        # (queue q's desc-gen runs on Q7 core pair q -> 4-way parallel).
        for g in range(NGATH):
            nc.gpsimd.dma_gather(
                c_t[:, g * GBLK * D:(g + 1) * GBLK * D].rearrange(
                    "p (j d) -> p j d", d=D),
                centers[:],
                lab_t[:, g * GBLK * (P // 16):(g + 1) * GBLK * (P // 16)],
                GBLK * P, GBLK * P, D, queue_num=g,
            ).then_inc(G[g], 16)

        # ---- Scalar/ACT: x half 1 on its HWDGE ring, squares ----
        # A events: 1=xx h1, 2=xx h2, 3=mn h1, 4=mn h2, 4+i+1 = chunk i acc
        # x + ACT tables load during the IRAM-load window (6.5-16us) so the
        # gather drains (16us on) get the DMA system to themselves.
        nc.scalar.dma_start(x_t[:, :H], xr[:, :H]).then_inc(X1, 16)
        # Dummy sqrt (scale=0, bias=1 -> sqrt(1)) pulls the ACT table load
        # ahead of the real sqrts; mn is rewritten below.
        nc.scalar.activation(mn[:, :1], mn[:, :1],
                             mybir.ActivationFunctionType.Sqrt,
                             bias=1.0, scale=0.0)
        nc.scalar.wait_ge(X1, 16)
        nc.scalar.square(xx[:, :H], x_t[:, :H]).then_inc(A, 1)
        nc.scalar.wait_ge(X2, 16)
        nc.scalar.square(xx[:, H:], x_t[:, H:]).then_inc(A, 1)
        nc.scalar.wait_ge(V, 1)
        nc.scalar.sqrt(mn[:, :J // 2], sx[:, :J // 2]).then_inc(A, 1)
        nc.scalar.wait_ge(V, 2)
        nc.scalar.sqrt(mn[:, J // 2:], sx[:, J // 2:]).then_inc(A, 1)
        for k in range(NGATH):
            f0, fn = k * GBLK * D, GBLK * D
            nc.scalar.wait_ge(V, 7 + k)
            nc.scalar.activation(c_t[:, f0:f0 + fn], c_t[:, f0:f0 + fn],
                                 mybir.ActivationFunctionType.Square,
                                 accum_out=acc[:, k:k + 1]).then_inc(A, 1)

        # ---- Vector/DVE ----
        # V events: 1=sx h1, 2=sx h2, 3=inv h1, 4=inv h2, 5=nx h1, 6=nx h2,
        # 6+i+1 = chunk i sub done
        def half(t, h):
            return t[:, h * H:(h + 1) * H].rearrange("p (j d) -> p j d", d=D)

        for h in range(2):
            nc.vector.wait_ge(A, 1 + h)
            nc.vector.reduce_sum(sx[:, h * J // 2:(h + 1) * J // 2],
                                 half(xx, h), axis=mybir.AxisListType.X
                                 ).then_inc(V, 1)
        for h in range(2):
            nc.vector.wait_ge(A, 3 + h)
            nc.vector.reciprocal(inv[:, h * J // 2:(h + 1) * J // 2],
                                 mn[:, h * J // 2:(h + 1) * J // 2]
                                 ).then_inc(V, 1)
        iap = inv[:]
        for h in range(2):
            nc.vector.wait_ge(V, 3 + h)
            ib = bass.AP(tensor=iap.tensor,
                         offset=iap.offset + h * (J // 2),
                         ap=[list(iap.ap[0]), [1, J // 2], [0, D]])
            nc.vector.tensor_tensor(out=half(nx, h), in0=half(x_t, h),
                                    in1=ib, op=mybir.AluOpType.mult
                                    ).then_inc(V, 1)
        nc.vector.wait_ge(V, 6)
        for k in range(NGATH):
            f0, fn = k * GBLK * D, GBLK * D
            nc.vector.wait_ge(G[k], 16)
            nc.vector.tensor_sub(c_t[:, f0:f0 + fn], nx[:, f0:f0 + fn],
                                 c_t[:, f0:f0 + fn]).then_inc(V, 1)

    nc.compile()
    return nc


def _get_nc():
    if "nc" not in _CACHE:
        _CACHE["nc"] = _build()
    return _CACHE["nc"]


def _prep_labels(lab_shard):
    """int16 idx layout for dma_gather: gather slot i = j*128+p must hold
    the label of x row p*16+j (so dst[i%128, i//128] aligns with x_t);
    then wrap slots into 16 partitions (idxs[c, s] = slot s*16+c) and
    replicate for the 8 Q7 core groups."""
    slots = lab_shard.reshape(P, J).T.reshape(-1)          # slot j*128+p
    wrapped = slots.reshape(ROWS // 16, 16).T              # [16, ROWS/16]
    return np.ascontiguousarray(
        np.tile(wrapped, (8, 1)).astype(np.int16))         # [128, ROWS/16]


def _run(x, labels, centers, trace=False):
    from concourse.bass_utils import run_bass_kernel_spmd

    x = np.ascontiguousarray(np.asarray(x, dtype=np.float32))
    labels = np.asarray(labels).astype(np.int16)
    centers = np.ascontiguousarray(np.asarray(centers, dtype=np.float32))

    in_maps = []
    for i in range(N_CORES):
        in_maps.append({
            "x": x[i * ROWS:(i + 1) * ROWS],
            "labels": _prep_labels(labels[i * ROWS:(i + 1) * ROWS]),
            "centers": centers,
        })
    res = run_bass_kernel_spmd(_get_nc(), in_maps,
                               core_ids=list(range(N_CORES)), trace=trace)
    total = np.float64(0.0)
    for r in res.results:
        total += np.float64(r["out"].sum(dtype=np.float64))
    loss = np.array(np.float32(total / B))
    return loss, res


def kernel(x, labels, centers):
    loss, _ = _run(x, labels, centers, trace=False)
    return loss


# revision 25
# speedup vs baseline: 1.0708x; 1.0708x over previous
"""CenterLoss kernel for Trainium2 (8 NeuronCores, data-parallel over batch).

loss = mean_i( ||nx_i||^2 + ||c_{l_i}||^2 - 2*nx_i.c_{l_i} )
     = mean_i( ||nx_i - c_{l_i}||^2 ),  nx_i = x_i / max(||x_i||, EPS)

The reference's (batch, num_classes) distmat is masked down to one column
per row, so only a gather of centers[labels] is needed (memory regime).

Sharding: batch 16384 -> 8 cores x 2048 rows, centers replicated. x row
p*16+j of a core's shard lives at SBUF partition p, free block j.
The centers gather uses InstDMAGatherAnt (gpsimd dma_gather), which is
Q7 descriptor-generation bound (~8.5ns/row on one tx/rx core pair; a
hot-labels probe showed no DRAM-locality sensitivity). Queue q's
desc-gen runs on Q7 core pair q (the plain indirect-DMA path is frozen
to pair 0), so the gather is split across all 4 SWDGE queues; HW
confirmed the pairs run ahead and overlap. The mlp library's ~6us IRAM
load is triggered by the FIRST Q7-executed op after the reload and
contends with concurrent HBM DMAs (10us when racing the x load), so a
tiny memset right after the reload triggers it immediately and signals
T; the x DMA waits for T and is split across the sync+scalar HWDGE
rings with the normalize pipeline run per half. Gather slot i writes
dst[i%128, i//128]; labels are permuted host-side (slot j*128+p =
label of x row p*16+j) and pre-wrapped into the int16 [16, n/16]
layout (replicated to all 8 Q7 core groups). Each gather has its own
semaphore (a shared sem races: its value can mix two gathers'
per-engine completions). A dummy sqrt preloads the ACT table. d = nx-c
and the Square+accumulate reduction are chunked per gather. Raw bacc
with manual semaphores. Each core returns per-partition partial sums;
the host combines.
"""

import numpy as np

B, C, D = 16384, 8192, 64
N_CORES = 8
ROWS = B // N_CORES        # 2048
P = 128
J = ROWS // P              # 16 blocks of D per partition
F = J * D                  # 1024 f32 per partition
NGATH = 4                  # one gather per SWDGE queue / Q7 pair
GBLK = J // NGATH          # 4 J-blocks per gather
H = F // 2                 # x pipeline half size

_CACHE = {}


def _build():
    from contextlib import ExitStack

    import concourse.bass as bass
    from concourse import bacc, library_config, mybir

    nc = bacc.Bacc("TRN2", target_bir_lowering=False, debug=False,
                   num_devices=N_CORES, dynamic_dma_scratch_size=131072,
                   num_swdge_queues=NGATH)
    f32 = mybir.dt.float32
    x = nc.dram_tensor("x", [ROWS, D], f32, kind="ExternalInput").ap()
    labels = nc.dram_tensor("labels", [P, ROWS // 16], mybir.dt.int16,
                            kind="ExternalInput").ap()
    centers = nc.dram_tensor("centers", [C, D], f32,
                             kind="ExternalInput").ap()
    out = nc.dram_tensor("out", [P, NGATH], f32, kind="ExternalOutput").ap()

    with ExitStack() as ctx:
        def sb(n, s, dt=f32):
            return ctx.enter_context(nc.sbuf_tensor(n, s, dt))
        lab_t = sb("lab_t", [P, ROWS // 16], mybir.dt.int16)
        tw = sb("tw", [P, 1])
        x_t = sb("x_t", [P, F])
        c_t = sb("c_t", [P, F])
        xx = sb("xx", [P, F])
        sx = sb("sx", [P, J])
        mn = sb("mn", [P, J])
        inv = sb("inv", [P, J])
        nx = sb("nx", [P, F])
        acc = sb("acc", [P, NGATH])
        T = ctx.enter_context(nc.semaphore("Tsem"))
        L = ctx.enter_context(nc.semaphore("Lsem"))
        X1 = ctx.enter_context(nc.semaphore("X1sem"))
        X2 = ctx.enter_context(nc.semaphore("X2sem"))
        G = [ctx.enter_context(nc.semaphore(f"Gsem{g}")) for g in range(NGATH)]
        A = ctx.enter_context(nc.semaphore("Asem"))   # ACT-produced events
        V = ctx.enter_context(nc.semaphore("Vsem"))   # DVE-produced events

        xr = x.rearrange("(p j) d -> p (j d)", p=P)

        # ---- Sync: labels in, x half 2 (T-gated past the IRAM load), out --
        nc.sync.dma_start(lab_t[:], labels[:]).then_inc(L, 16)
        nc.sync.wait_ge(T, 1)
        nc.sync.dma_start(x_t[:, H:], xr[:, H:]).then_inc(X2, 16)
        nc.sync.wait_ge(A, 4 + NGATH)
        # No final wait on the out DMA: the bacc epilogue's engine drain
        # covers its completion, overlapping the exit barrier ladder.
        nc.sync.dma_start(out, acc[:]).then_inc(L, 16)

        # ---- GpSimd: trigger the mlp IRAM load, then the gathers ----
        nc.gpsimd.load_library(library_config.mlp)
        # First Q7-executed op after the reload faults the library blob in
        # (~2.5us reload + ~6.4us load); doing it with a memset starts the
        # load ~1.5us earlier than the first gather's own pop would. T then
        # releases the x DMAs, keeping them off both the load and the
        # gather-drain windows' bandwidth as much as possible.
        nc.gpsimd.memset(tw[:], 0.0).then_inc(T, 1)
        nc.gpsimd.wait_ge(L, 16)
        # gather g covers slots/blocks [g*GBLK, (g+1)*GBLK) on SWDGE queue g
        # (queue q's desc-gen runs on Q7 core pair q -> 4-way parallel).
        for g in range(NGATH):
            nc.gpsimd.dma_gather(
                c_t[:, g * GBLK * D:(g + 1) * GBLK * D].rearrange(
                    "p (j d) -> p j d", d=D),
                centers[:],
                lab_t[:, g * GBLK * (P // 16):(g + 1) * GBLK * (P // 16)],
                GBLK * P, GBLK * P, D, queue_num=g,
            ).then_inc(G[g], 16)

        # ---- Scalar/ACT: x half 1 on its HWDGE ring, squares ----
        # A events: 1=xx h1, 2=xx h2, 3=mn h1, 4=mn h2, 4+i+1 = chunk i acc
        # Dummy sqrt first: both ACT tables DMA in at ~6.5-9us, before the
        # IRAM load triggers and well clear of the gather-drain window.
        nc.scalar.activation(mn[:, :1], mn[:, :1],
                             mybir.ActivationFunctionType.Sqrt,
                             bias=1.0, scale=0.0)
        nc.scalar.wait_ge(T, 1)
        nc.scalar.dma_start(x_t[:, :H], xr[:, :H]).then_inc(X1, 16)
        nc.scalar.wait_ge(X1, 16)
        nc.scalar.square(xx[:, :H], x_t[:, :H]).then_inc(A, 1)
        nc.scalar.wait_ge(X2, 16)
        nc.scalar.square(xx[:, H:], x_t[:, H:]).then_inc(A, 1)
        nc.scalar.wait_ge(V, 1)
        nc.scalar.sqrt(mn[:, :J // 2], sx[:, :J // 2]).then_inc(A, 1)
        nc.scalar.wait_ge(V, 2)
        nc.scalar.sqrt(mn[:, J // 2:], sx[:, J // 2:]).then_inc(A, 1)
        for k in range(NGATH):
            f0, fn = k * GBLK * D, GBLK * D
            nc.scalar.wait_ge(V, 7 + k)
            nc.scalar.activation(c_t[:, f0:f0 + fn], c_t[:, f0:f0 + fn],
                                 mybir.ActivationFunctionType.Square,
                                 accum_out=acc[:, k:k + 1]).then_inc(A, 1)

        # ---- Vector/DVE ----
        # V events: 1=sx h1, 2=sx h2, 3=inv h1, 4=inv h2, 5=nx h1, 6=nx h2,
        # 6+i+1 = chunk i sub done
        def half(t, h):
            return t[:, h * H:(h + 1) * H].rearrange("p (j d) -> p j d", d=D)

        for h in range(2):
            nc.vector.wait_ge(A, 1 + h)
            nc.vector.reduce_sum(sx[:, h * J // 2:(h + 1) * J // 2],
                                 half(xx, h), axis=mybir.AxisListType.X
                                 ).then_inc(V, 1)
        for h in range(2):
            nc.vector.wait_ge(A, 3 + h)
            nc.vector.reciprocal(inv[:, h * J // 2:(h + 1) * J // 2],
                                 mn[:, h * J // 2:(h + 1) * J // 2]
                                 ).then_inc(V, 1)
        iap = inv[:]
        for h in range(2):
            nc.vector.wait_ge(V, 3 + h)
            ib = bass.AP(tensor=iap.tensor,
                         offset=iap.offset + h * (J // 2),
                         ap=[list(iap.ap[0]), [1, J // 2], [0, D]])
            nc.vector.tensor_tensor(out=half(nx, h), in0=half(x_t, h),
                                    in1=ib, op=mybir.AluOpType.mult
                                    ).then_inc(V, 1)
        nc.vector.wait_ge(V, 6)
        for k in range(NGATH):
            f0, fn = k * GBLK * D, GBLK * D
            nc.vector.wait_ge(G[k], 16)
            nc.vector.tensor_sub(c_t[:, f0:f0 + fn], nx[:, f0:f0 + fn],
                                 c_t[:, f0:f0 + fn]).then_inc(V, 1)

    nc.compile()
    return nc


def _get_nc():
    if "nc" not in _CACHE:
        _CACHE["nc"] = _build()
    return _CACHE["nc"]


def _prep_labels(lab_shard):
    """int16 idx layout for dma_gather: gather slot i = j*128+p must hold
    the label of x row p*16+j (so dst[i%128, i//128] aligns with x_t);
    then wrap slots into 16 partitions (idxs[c, s] = slot s*16+c) and
    replicate for the 8 Q7 core groups."""
    slots = lab_shard.reshape(P, J).T.reshape(-1)          # slot j*128+p
    wrapped = slots.reshape(ROWS // 16, 16).T              # [16, ROWS/16]
    return np.ascontiguousarray(
        np.tile(wrapped, (8, 1)).astype(np.int16))         # [128, ROWS/16]


def _run(x, labels, centers, trace=False):
    from concourse.bass_utils import run_bass_kernel_spmd

    x = np.ascontiguousarray(np.asarray(x, dtype=np.float32))
    labels = np.asarray(labels).astype(np.int16)
    centers = np.ascontiguousarray(np.asarray(centers, dtype=np.float32))

    in_maps = []
    for i in range(N_CORES):
        in_maps.append({
            "x": x[i * ROWS:(i + 1) * ROWS],
            "labels": _prep_labels(labels[i * ROWS:(i + 1) * ROWS]),
            "centers": centers,
        })
    res = run_bass_kernel_spmd(_get_nc(), in_maps,
                               core_ids=list(range(N_CORES)), trace=trace)
    total = np.float64(0.0)
    for r in res.results:
        total += np.float64(r["out"].sum(dtype=np.float64))
    loss = np.array(np.float32(total / B))
    return loss, res


def kernel(x, labels, centers):
    loss, _ = _run(x, labels, centers, trace=False)
    return loss
